# revision 19
# baseline (speedup 1.0000x reference)
"""Trainium2 Bass kernel for EncoderDecoderLSTMCell.

Model (reference semantics):
  encoded = input_seq @ W_enc.T + b_enc                    [B, T, 256]
  512 past LSTM steps:  gates = enc_t @ W_ih.T + b_ih + h @ W_hh.T + b_hh
  128 future steps:     u = h @ W_fenc.T + b_fenc; gates = u @ W_ih.T + ... + h @ W_hh.T + b_hh
  out = hs @ W_dec.T + b_dec                               [B, 640, 64]

Strategy: data-parallel over batch (128 -> 16 per core, 8 cores), everything
else local. Host folds weights:
  past:   gates = x_t @ (W_ih W_enc).T + h @ W_hh.T + (W_ih b_enc + b_ih + b_hh)
  future: gates = h @ (W_hh + W_ih W_fenc).T + (W_ih b_fenc + b_ih + b_hh)
On-device layout is fully transposed: state h.T/c.T live as [128 hid-part,
(ktile, batch)] so the recurrent matmul runs with constant fp16 weight
stationaries (gates.T = W @ h.T, 16 Mtiles x 4 Ktiles of [128,128]) and all
elementwise work is 128-partition dense.

Gate Mtile order is (g, i, f, o): the per-step critical path is the c-chain
(needs g,i early, f next, o only at the very end), so sigmoid of each gate
block is issued as soon as its 4 Mtiles of matmuls retire, overlapping the
ACT/DVE/Pool chain with the remaining weight loads on PE (PE is ~98% busy:
~51ns per self-loading matmul instruction, dtype-independent -- fp8
stationaries measured no faster, so weights stay fp16). t1 runs on DVE in
parallel with t2 on Pool. The x-projection is interleaved one Mtile per
step into the PE tail bubble (a 16-matmul burst at chunk boundaries would
stall the critical path), and the decoder is interleaved into the future
steps the same way, one group per step.
"""

import numpy as np

F_IN, REC_IP, HID, F_OUT = 64, 256, 512, 64
B_FULL, T_PAST, T_FUT = 128, 512, 128
N_CORES = 8
BC = B_FULL // N_CORES  # 16
NM, NK = 16, 4  # gate Mtiles (2048/128), hid Ktiles (512/128)
CH = 16  # x-proj chunk size (timesteps per staging buffer)

_CACHE = {}


def _gate_perm_rows():
    # torch gate order (i, f, g, o) -> our Mtile block order (g, i, f, o)
    return np.concatenate(
        [np.arange(1024, 1536), np.arange(0, 512), np.arange(512, 1024), np.arange(1536, 2048)]
    )


def _build_program(t_past, t_fut, dbg=False, reps=1):
    import concourse.bacc as bacc
    import concourse.bass as bass
    from concourse.tile import TileContext
    from concourse import mybir

    f32 = mybir.dt.float32
    bf16 = mybir.dt.float16  # fp16: same PE rate, 8x finer mantissa than bf16
    Sig = mybir.ActivationFunctionType.Sigmoid

    t_tot = t_past + t_fut
    nc = bacc.Bacc(
        "TRN2", target_bir_lowering=False, debug=False, num_devices=N_CORES
    )

    xT = nc.dram_tensor("xT", [F_IN, t_past * BC], bf16, kind="ExternalInput")
    wp = nc.dram_tensor("wp", [128, NM * NK * 128], bf16, kind="ExternalInput")
    wf = nc.dram_tensor("wf", [128, NM * NK * 128], bf16, kind="ExternalInput")
    wxg = nc.dram_tensor("wxg", [F_IN, NM * 128], bf16, kind="ExternalInput")
    wdec = nc.dram_tensor("wdec", [128, NK * F_OUT], bf16, kind="ExternalInput")
    bpast = nc.dram_tensor("bpast", [128, NM], f32, kind="ExternalInput")
    bfut = nc.dram_tensor("bfut", [128, NM * BC], bf16, kind="ExternalInput")
    idin = nc.dram_tensor("idin", [128, 128], bf16, kind="ExternalInput")
    out_d = nc.dram_tensor("out", [BC, t_tot, F_OUT], f32, kind="ExternalOutput")

    ch_sz = min(CH, t_past)
    assert t_past % ch_sz == 0
    n_ch = t_past // ch_sz

    with TileContext(nc) as tc:
        with (
            tc.tile_pool(name="const", bufs=1) as cpool,
            tc.tile_pool(name="state", bufs=1) as spool,
            tc.tile_pool(name="gx", bufs=3) as gxpool,
            tc.tile_pool(name="gtmp", bufs=3) as gtmp,
            tc.tile_pool(name="dstage", bufs=4) as dstage,
            tc.tile_pool(name="pscan", bufs=2, space="PSUM") as pscan,
            tc.tile_pool(name="pxp", bufs=2, space="PSUM") as pxp,
            tc.tile_pool(name="pdec", bufs=2, space="PSUM") as pdec,
        ):
            # ---- resident constants ----
            wp_sb = cpool.tile([128, NM * NK * 128], bf16)
            nc.sync.dma_start(out=wp_sb, in_=wp[:, :])
            wf_sb = cpool.tile([128, NM * NK * 128], bf16)
            nc.sync.dma_start(out=wf_sb, in_=wf[:, :])
            wxg_sb = cpool.tile([F_IN, NM * 128], bf16)
            nc.sync.dma_start(out=wxg_sb, in_=wxg[:, :])
            wdec_sb = cpool.tile([128, NK * F_OUT], bf16)
            nc.sync.dma_start(out=wdec_sb, in_=wdec[:, :])
            bpast_sb = cpool.tile([128, NM], f32)
            nc.sync.dma_start(out=bpast_sb, in_=bpast[:, :])
            bfut_sb = cpool.tile([128, NM * BC], bf16)
            nc.sync.dma_start(out=bfut_sb, in_=bfut[:, :])
            xT_sb = cpool.tile([F_IN, t_past * BC], bf16)
            nc.sync.dma_start(out=xT_sb, in_=xT[:, :])

            # ---- state ----
            # hs: h.T history, col (t, k, b) = t*64 + k*16 + b; t=0 is h0=0
            hs = spool.tile([128, (t_tot + 1) * HID // 8], bf16)
            c_st = spool.tile([128, 64], f32)

            # ---- x-projection, one Mtile of one chunk at a time ----
            def emit_xproj_mtile(stage, c, m):
                ps = pxp.tile([128, ch_sz * BC], f32)
                nc.tensor.matmul(
                    ps,
                    wxg_sb[:, m * 128 : (m + 1) * 128],
                    xT_sb[:, c * ch_sz * BC : (c + 1) * ch_sz * BC],
                    start=True,
                    stop=True,
                )
                # stage[:, tl*256 + m*16 + b] = ps[:, tl*16 + b] + bias_m
                dst = stage[:].rearrange("p (tl mm b) -> p tl mm b", mm=NM, b=BC)[
                    :, :, m, :
                ]
                # gpsimd cannot read PSUM; alternate DVE / ACT
                if m % 2 == 0:
                    nc.vector.tensor_scalar_add(
                        dst, ps[:].rearrange("p (tl b) -> p tl b", b=BC),
                        bpast_sb[:, m : m + 1],
                    )
                else:
                    nc.scalar.activation(
                        dst,
                        ps[:].rearrange("p (tl b) -> p tl b", b=BC),
                        mybir.ActivationFunctionType.Identity,
                        bias=bpast_sb[:, m : m + 1],
                    )

            def new_stage():
                return gxpool.tile([128, ch_sz * NM * BC], bf16, tag="gxstage",
                                   name="stage")

            def emit_xproj(c):
                stage = new_stage()
                for m in range(NM):
                    emit_xproj_mtile(stage, c, m)
                return stage

            # ---- one LSTM step ----
            # All-sigmoid chain: tanh(x) = 2*sig(2x)-1, with the 2x folded
            # into g-gate weights, c2 := 2c, h' := h/2 (W_hh, W_dec
            # pre-scaled on host). Single LUT -> no ACT table switches.
            # Gate blocks in PSUM cols: g [0:64], i [64:128], f [128:192],
            # o [192:256]; sigmoid of each block issues as soon as its
            # matmuls retire so the chain overlaps the o-gate weight loads.
            # gx + bias are DVE-copied into the PSUM tile one step ahead
            # (PE was 100% busy; the id-matmul injection cost 160ns/step of
            # PE time, the DVE copy rides in its idle window)
            gx_ps = {}

            def prefetch_gx(t, gx_slice):
                g_ps = pscan.tile([128, NM * BC], f32, tag="gps", name="g_ps")
                nc.vector.tensor_copy(out=g_ps, in_=gx_slice)
                gx_ps[t] = g_ps

            def emit_step(t, w_sb):
                g_ps = gx_ps.pop(t)
                sig = gtmp.tile([128, NM * BC], f32, tag="sig")
                t1 = gtmp.tile([128, 64], f32, tag="t1")
                t2 = gtmp.tile([128, 64], f32, tag="t2")
                # NOTE: accumulation groups must be contiguous (m-outer):
                # interleaving k-outer across column slices gives wrong PSUM
                # accumulation on HW.
                for m in range(NM):
                    ps = g_ps[:, m * BC : (m + 1) * BC]
                    for k in range(NK):
                        nc.tensor.matmul(
                            ps,
                            w_sb[:, (m * NK + k) * 128 : (m * NK + k + 1) * 128],
                            hs[:, t * 64 + k * 16 : t * 64 + (k + 1) * 16],
                            start=False,
                            stop=(k == NK - 1),
                            skip_group_check=True,
                        )
                    if m == 7:
                        # g, i blocks done: t1 = (sig_g - 0.5) * sig_i on DVE
                        # (scalar_tensor_tensor is not supported on Pool)
                        nc.scalar.activation(sig[:, 0:128], g_ps[:, 0:128], Sig)
                        nc.vector.scalar_tensor_tensor(
                            out=t1, in0=sig[:, 0:64], scalar=0.5,
                            in1=sig[:, 64:128],
                            op0=mybir.AluOpType.subtract,
                            op1=mybir.AluOpType.mult,
                        )
                    elif m == 11:
                        # f block done: t2 = sig_f * c2_prev on Pool (parallel
                        # with t1 on DVE)
                        nc.scalar.activation(sig[:, 128:192], g_ps[:, 128:192], Sig)
                        nc.gpsimd.tensor_mul(t2, sig[:, 128:192], c_st)
                # c2 = 4*t1 + t2
                nc.vector.scalar_tensor_tensor(
                    out=c_st, in0=t1, scalar=4.0, in1=t2,
                    op0=mybir.AluOpType.mult, op1=mybir.AluOpType.add,
                )
                sc = gtmp.tile([128, 64], f32, tag="sc")
                nc.scalar.activation(sc, c_st, Sig)
                # sig_o after sc in the ACT queue: o matmuls retire during the
                # c-chain, so this slots in right behind sc.
                nc.scalar.activation(sig[:, 192:256], g_ps[:, 192:256], Sig)
                nc.vector.scalar_tensor_tensor(
                    out=hs[:, (t + 1) * 64 : (t + 2) * 64],
                    in0=sc, scalar=0.5, in1=sig[:, 192:256],
                    op0=mybir.AluOpType.subtract, op1=mybir.AluOpType.mult,
                )

            # ---- past scan, pipelined with x-proj ----
            for _rep in range(reps):
              nc.vector.memset(hs[:, 0:64], 0.0)
              nc.vector.memset(c_st, 0.0)
              stages = {}
              stages[0] = emit_xproj(0)
              if n_ch > 1:
                  stages[1] = emit_xproj(1)
              assert ch_sz == NM
              stage_views = {}

              def gx_for(t):
                  if t < t_past:
                      c, tl = divmod(t, ch_sz)
                      return stage_views[c][:, tl, :]
                  return bfut_sb[:, :]

              for c in (0, 1):
                  if c in stages:
                      stage_views[c] = stages[c][:].rearrange(
                          "p (tl x) -> p tl x", x=NM * BC)
              prefetch_gx(0, gx_for(0))
              for c in range(n_ch):
                  stages.pop(c)
                  tgt = c + 2
                  if tgt < n_ch:
                      stages[tgt] = new_stage()
                      stage_views[tgt] = stages[tgt][:].rearrange(
                          "p (tl x) -> p tl x", x=NM * BC)
                  for tl in range(ch_sz):
                      t = c * ch_sz + tl
                      if t + 1 < t_past + t_fut:
                          prefetch_gx(t + 1, gx_for(t + 1))
                      emit_step(t, wp_sb)
                      # one x-proj Mtile of chunk c+2 per step: fills the PE
                      # tail bubble without the 16-matmul burst at chunk
                      # boundaries delaying the critical path
                      if tgt < n_ch:
                          emit_xproj_mtile(stages[tgt], tgt, tl)

              # ---- decode group: out[b, t, f] = h_{t+1} @ W_dec.T ----
              TG = 8  # timesteps per decode group -> (t,b) fills 128 partitions
              hs_v = hs[:].rearrange("p (t x) -> p t x", x=64)

              def emit_decode_group(g):
                  ps = pdec.tile([128, F_OUT], f32)
                  for k in range(NK):
                      lhs = dstage.tile([128, TG * 16], bf16, tag="declhs")
                      eng = nc.vector if k % 2 == 0 else nc.gpsimd
                      eng.tensor_copy(
                          out=lhs[:].rearrange("p (t x) -> p t x", x=16),
                          in_=hs_v[:, 1 + g * TG : 1 + (g + 1) * TG, k * 16 : (k + 1) * 16],
                      )
                      nc.tensor.matmul(
                          ps,
                          lhs,
                          wdec_sb[:, k * F_OUT : (k + 1) * F_OUT],
                          start=(k == 0),
                          stop=(k == NK - 1),
                      )
                  st = dstage.tile([128, F_OUT], f32)
                  if g % 2 == 0:
                      nc.vector.tensor_copy(out=st, in_=ps)
                  else:
                      nc.scalar.activation(
                          out=st, in_=ps, func=mybir.ActivationFunctionType.Copy
                      )
                  oap = out_d.ap()
                  dst = bass.AP(
                      tensor=oap.tensor,
                      offset=g * TG * F_OUT,
                      ap=[[F_OUT, TG], [t_tot * F_OUT, BC], [1, F_OUT]],
                  )
                  nc.sync.dma_start(out=dst, in_=st)

              # ---- future steps, decode interleaved into the PE tail
              # bubbles (group g during future step g + t_fut - n_groups is
              # always past the step that writes its last h) ----
              n_groups = t_tot // TG
              dec_start = t_fut - n_groups
              assert dec_start >= 0
              for j in range(t_fut):
                  t = t_past + j
                  if t + 1 < t_past + t_fut:
                      prefetch_gx(t + 1, gx_for(t + 1))
                  emit_step(t, wf_sb)
                  if j >= dec_start:
                      emit_decode_group(j - dec_start)

    nc.compile()
    return nc


def _prep_host(inputs, t_past=T_PAST):
    """Fold weights/biases and build per-core input maps."""
    bf16 = np.float16
    x = np.asarray(inputs["input_seq"], np.float32)
    W_enc = np.asarray(inputs["W_enc"], np.float64)
    b_enc = np.asarray(inputs["b_enc"], np.float64)
    W_ih = np.asarray(inputs["W_ih"], np.float64)
    b_ih = np.asarray(inputs["b_ih"], np.float64)
    W_hh = np.asarray(inputs["W_hh"], np.float64)
    b_hh = np.asarray(inputs["b_hh"], np.float64)
    W_fenc = np.asarray(inputs["W_fenc"], np.float64)
    b_fenc = np.asarray(inputs["b_fenc"], np.float64)
    W_dec = np.asarray(inputs["W_dec"], np.float64)

    perm = _gate_perm_rows()
    W_xg = (W_ih @ W_enc)[perm]  # [2048, 64]
    b_past = (W_ih @ b_enc + b_ih + b_hh)[perm]  # [2048]
    W_hh_p = W_hh[perm]  # [2048, 512]
    W_fut = (W_hh + W_ih @ W_fenc)[perm]  # [2048, 512]
    b_fut = (W_ih @ b_fenc + b_ih + b_hh)[perm]
    # all-sigmoid rescaling: g-gate rows x2 (tanh(x)=2sig(2x)-1), then the
    # h-input side x2 because the device stores h' = h/2; W_dec x2 likewise.
    gsc = np.ones((2048, 1)); gsc[0:512] = 2.0  # g-gate rows (block order g,i,f,o)
    W_xg = W_xg * gsc
    b_past = b_past * gsc[:, 0]
    W_hh_p = W_hh_p * gsc * 2.0
    W_fut = W_fut * gsc * 2.0
    b_fut = b_fut * gsc[:, 0]
    W_dec = W_dec * 2.0

    def stationaries(Wm):  # [2048, 512] -> [128, NM*NK*128]
        out = np.empty((128, NM * NK * 128), np.float32)
        for m in range(NM):
            for k in range(NK):
                out[:, (m * NK + k) * 128 : (m * NK + k + 1) * 128] = Wm[
                    m * 128 : (m + 1) * 128, k * 128 : (k + 1) * 128
                ].T
        return out

    wp_np = stationaries(W_hh_p).astype(bf16)
    wf_np = stationaries(W_fut).astype(bf16)
    wxg_np = np.empty((F_IN, NM * 128), np.float32)
    for m in range(NM):
        wxg_np[:, m * 128 : (m + 1) * 128] = W_xg[m * 128 : (m + 1) * 128, :].T
    wxg_np = wxg_np.astype(bf16)
    wdec_np = np.empty((128, NK * F_OUT), np.float32)
    for k in range(NK):
        wdec_np[:, k * F_OUT : (k + 1) * F_OUT] = W_dec[:, k * 128 : (k + 1) * 128].T
    wdec_np = wdec_np.astype(bf16)

    bpast_np = b_past.reshape(NM, 128).T.astype(np.float32).copy()  # [128, NM]
    bfut_np = np.repeat(b_fut.reshape(NM, 128).T[:, :, None], BC, axis=2).reshape(
        128, NM * BC
    ).astype(bf16)
    idin_np = np.eye(128, dtype=bf16)

    in_maps = []
    for ci in range(N_CORES):
        xs = x[ci * BC : (ci + 1) * BC, :t_past]  # [16, t_past, 64]
        xT_np = np.ascontiguousarray(xs.transpose(2, 1, 0).reshape(F_IN, -1)).astype(
            bf16
        )  # col t*16+b
        in_maps.append(
            {
                "xT": xT_np,
                "wp": wp_np,
                "wf": wf_np,
                "wxg": wxg_np,
                "wdec": wdec_np,
                "bpast": bpast_np,
                "bfut": bfut_np,
                "idin": idin_np,
            }
        )
    return in_maps


def kernel(**inputs):
    from concourse import bass_utils

    fut = int(np.asarray(inputs.get("future_n", T_FUT)))
    assert fut == T_FUT, f"kernel compiled for future_n={T_FUT}, got {fut}"

    key = (T_PAST, T_FUT)
    if key not in _CACHE:
        _CACHE[key] = _build_program(T_PAST, T_FUT)
    nc = _CACHE[key]

    in_maps = _prep_host(inputs)
    res = bass_utils.run_bass_kernel_spmd(nc, in_maps, core_ids=list(range(N_CORES)))
    out = np.concatenate([r["out"] for r in res.results], axis=0)
    return out.astype(np.float32)


if __name__ == "__main__":
    pass
